# revision 4
# baseline (speedup 1.0000x reference)
"""Cross-attention layer (vision<->text) on 8 Trainium2 NeuronCores.

Problem: B=16, Sv=St=1024, D=1024, fp32.
  q = vision @ Wq.T + bq            [B,Sv,D]
  k = text   @ Wk.T + bk            [B,St,D]
  v = text   @ Wv.T + bv            [B,St,D]
  scores = q @ k.T / sqrt(D)        [B,Sv,St]
  attn = softmax(scores, -1)
  cross_vision = attn @ v           [B,Sv,D]
  cross_text   = attn.T @ vision    [B,St,D]

Sharding: pure data-parallel over batch, 2 items per core, no collectives.

Per-core kernel design (per batch item):
  - Host pre-transposes weights: wqt = Wq.T/sqrt(D) [d,e], wkt = Wk.T, wvt = Wv.T.
    The 1/sqrt(D) is folded into wqt/bq. bv is added on the host after gather
    (attn rows sum to 1, so attn @ (v0 + bv) = attn @ v0 + bv, exact).
  - On-chip PE transposes build VT[d,s] and TT[d,t] from the natural-layout
    activations, half the seq dim at a time (SBUF economy).
  - QT[e,s] = wqt.T @ VT, KT[e,t] = wkt.T @ TT (weight tile stationary),
    Vv[t,d'] = TT.T @ wvt (TT tile stationary). All matmuls run as float32r
    (fp32 bits, PE truncates to ~fp22: full-rate at N>=512, ~2^-12 rel err).
  - S[s,t] = QT.T @ KT per 128-row s-tile; E = exp(S) straight out of PSUM on
    the ACT engine with accum_out producing row sums (scores are O(+-6), no
    max-subtraction needed for fp32 exp). rinv = 1/rowsum.
  - cross_vision s-tile: PE-transpose E row-block -> ET blocks, then
    CV = ET.T @ Vv accumulated over t-tiles, scaled by rinv at PSUM evac.
  - E is then scaled in-place by rinv (making attn rows), and
    cross_text = E.T @ V accumulated over s-tiles with raw V streamed back in.
"""

import sys

import numpy as np

if "/opt/trn_rl_repo" not in sys.path:
    sys.path.insert(0, "/opt/trn_rl_repo")

import concourse.bass as bass
import concourse.tile as tile
from concourse import bacc
from concourse import mybir
from concourse.masks import make_identity

P = 128
B, SEQ, DIM = 16, 1024, 1024
N_CORES = 8
BPC = B // N_CORES  # batch items per core
NT = DIM // P  # 8 tiles of 128 along d/e
F32 = mybir.dt.float32
F32R = mybir.dt.float32r
AF = mybir.ActivationFunctionType
H = 512  # half of a seq dim / PSUM-bank-sized chunk


def _emit(tc, ident, vis, txt, wqt, wkt, wvt, bq_sb, bk_sb, cv_d, ct_d, pools, b):
    nc = tc.nc
    (p_act, p_kt, p_qt, p_vv, p_e, p_etb, p_wc, p_wr, p_in, p_cvs, p_cts, p_rp,
     p_rv, pp_t, pp_mm) = pools

    # Tiles consumed by fp32r matmuls are declared float32r: the BIR verifier
    # requires fp32r matmul inputs to be produced as fp32r (DVE/ACT round on
    # write; DMA-fed weight tiles are fp32r end-to-end from fp32r DRAM).
    kt = p_kt.tile([P, NT, SEQ], F32R, name="kt", tag="kt")
    vv = p_vv.tile([P, NT, SEQ], F32R, name="vv", tag="vv")
    qt = p_qt.tile([P, NT, SEQ], F32R, name="qt", tag="qt")

    def prep_half(src_d, h):
        """Transpose 4 seq-row-tiles of src into actT[d_in, d_out, local_seq]."""
        actT = p_act.tile([P, NT, H], F32R, name="actT", tag="actT")
        for l in range(4):
            rt = h * 4 + l
            tin = p_in.tile([P, DIM], F32, name="tin", tag="xin")
            nc.sync.dma_start(out=tin, in_=src_d[b, rt * P:(rt + 1) * P, :])
            for dg in range(2):  # two 4-transpose groups, each one PSUM bank
                tp4 = pp_t.tile([P, 4, P], F32, name="tp4", tag="tp4")
                for j in range(4):
                    do = dg * 4 + j
                    nc.tensor.matmul(
                        tp4[:, j, :], tin[:, do * P:(do + 1) * P], ident,
                        is_transpose=True, start=(j == 0), stop=(j == 3),
                        skip_group_check=True,
                    )
                nc.vector.tensor_copy(actT[:, dg * 4:(dg + 1) * 4, l * P:(l + 1) * P], tp4)
        return actT

    def proj_half(w_d, bias_col, actT, out_sb, h, on_vector):
        """out_sb[:, eo, h*H:+H] = sum_do w[do,eo].T @ actT[:, do, :] (+bias)."""
        for eo in range(NT):
            wc = p_wc.tile([P, NT, P], F32R, name="wc", tag="wc")
            nc.sync.dma_start(
                out=wc,
                in_=w_d[:, eo * P:(eo + 1) * P].rearrange("(do di) e -> di do e", di=P),
            )
            ps = pp_mm.tile([P, H], F32, name="ps_proj", tag="mm")
            for do in range(NT):
                nc.tensor.matmul(ps, wc[:, do, :], actT[:, do, :],
                                 start=(do == 0), stop=(do == NT - 1))
            dst = out_sb[:, eo, h * H:(h + 1) * H]
            if on_vector:
                nc.vector.tensor_scalar_add(dst, ps, scalar1=bias_col[:, eo:eo + 1])
            else:
                nc.scalar.add(dst, ps, add=bias_col[:, eo:eo + 1])

    def vv_half(actT, h):
        """vv[t-tiles of half h, :] = sum_do TT[:,do,t].T @ wvt[do-rows, :]."""
        for dc in range(2):
            pss = [pp_mm.tile([P, H], F32, name=f"ps_vv{i}", tag="mm") for i in range(4)]
            for do in range(NT):
                wr = p_wr.tile([P, H], F32R, name="wr", tag="wr")
                nc.sync.dma_start(out=wr, in_=wvt[do * P:(do + 1) * P, dc * H:(dc + 1) * H])
                for l in range(4):
                    nc.tensor.matmul(pss[l], actT[:, do, l * P:(l + 1) * P], wr,
                                     start=(do == 0), stop=(do == NT - 1))
            for l in range(4):
                tt = h * 4 + l
                nc.scalar.copy(vv[:, tt, dc * H:(dc + 1) * H], pss[l])

    # ---- phases A-C: text -> TT -> KT, Vv (per t-half) ----
    for h in range(2):
        actT = prep_half(txt, h)
        proj_half(wkt, bk_sb, actT, kt, h, on_vector=False)
        vv_half(actT, h)

    # ---- phases D-E: vision -> VT -> QT (per s-half) ----
    for h in range(2):
        actV = prep_half(vis, h)
        proj_half(wqt, bq_sb, actV, qt, h, on_vector=True)

    # ---- phase F: scores, softmax, cross_vision (per s-tile) ----
    e_sb = p_e.tile([P, NT, SEQ], F32R, name="e_sb", tag="e")
    rinv = p_rv.tile([P, NT], F32, name="rinv", tag="rinv")
    for so in range(NT):
        rp = p_rp.tile([P, 2], F32, name="rp", tag="rp")
        for tc_ in range(2):
            ps = pp_mm.tile([P, H], F32, name="ps_s", tag="mm")
            for eo in range(NT):
                nc.tensor.matmul(ps, qt[:, eo, so * P:(so + 1) * P],
                                 kt[:, eo, tc_ * H:(tc_ + 1) * H],
                                 start=(eo == 0), stop=(eo == NT - 1))
            nc.scalar.activation(out=e_sb[:, so, tc_ * H:(tc_ + 1) * H], in_=ps,
                                 func=AF.Exp, accum_out=rp[:, tc_:tc_ + 1])
        rsum = p_rp.tile([P, 1], F32, name="rsum", tag="rsum")
        nc.vector.tensor_add(rsum, rp[:, 0:1], rp[:, 1:2])
        nc.vector.reciprocal(rinv[:, so:so + 1], rsum)

        # ET blocks for this s-tile (transpose the *unnormalized* E row-block)
        etb = p_etb.tile([P, NT, P], F32R, name="etb", tag="etb")
        for tg in range(2):
            tp4 = pp_t.tile([P, 4, P], F32, name="tp4e", tag="tp4")
            for j in range(4):
                tt = tg * 4 + j
                nc.tensor.matmul(tp4[:, j, :], e_sb[:, so, tt * P:(tt + 1) * P].bitcast(F32), ident,
                                 is_transpose=True, start=(j == 0), stop=(j == 3),
                                 skip_group_check=True)
            nc.vector.tensor_copy(etb[:, tg * 4:(tg + 1) * 4, :], tp4)

        # normalize this E row-block in place (for cross_text later)
        nc.vector.tensor_scalar_mul(e_sb[:, so, :], e_sb[:, so, :],
                                    scalar1=rinv[:, so:so + 1])

        # cross_vision[s-tile] = rinv * (ET.T @ Vv)
        cvs = p_cvs.tile([P, DIM], F32, name="cvs", tag="cvs")
        for dc in range(2):
            ps = pp_mm.tile([P, H], F32, name="ps_cv", tag="mm")
            for tt in range(NT):
                nc.tensor.matmul(ps, etb[:, tt, :], vv[:, tt, dc * H:(dc + 1) * H],
                                 start=(tt == 0), stop=(tt == NT - 1))
            nc.scalar.mul(cvs[:, dc * H:(dc + 1) * H], ps, mul=rinv[:, so:so + 1])
        nc.sync.dma_start(out=cv_d[b, so * P:(so + 1) * P, :], in_=cvs)

    # ---- phase H: cross_text = E'.T @ V (E' already rinv-scaled) ----
    for dc in range(2):
        for tg in range(2):
            pss = [pp_mm.tile([P, H], F32, name=f"ps_ct{i}", tag="mm") for i in range(4)]
            for so in range(NT):
                vt = p_in.tile([P, H], F32R, name="vt", tag="xin")
                nc.sync.dma_start(out=vt, in_=vis[b, so * P:(so + 1) * P, dc * H:(dc + 1) * H].bitcast(F32R))
                for i in range(4):
                    tt = tg * 4 + i
                    nc.tensor.matmul(pss[i], e_sb[:, so, tt * P:(tt + 1) * P], vt,
                                     start=(so == 0), stop=(so == NT - 1))
            for i in range(4):
                tt = tg * 4 + i
                cts = p_cts.tile([P, H], F32, name="cts", tag="cts")
                nc.vector.tensor_copy(cts, pss[i])
                nc.sync.dma_start(out=ct_d[b, tt * P:(tt + 1) * P, dc * H:(dc + 1) * H],
                                  in_=cts)


def build_nc():
    nc = bacc.Bacc("TRN2", target_bir_lowering=False, debug=False, num_devices=N_CORES)
    vis = nc.dram_tensor("vision", [BPC, SEQ, DIM], F32, kind="ExternalInput").ap()
    txt = nc.dram_tensor("text", [BPC, SEQ, DIM], F32, kind="ExternalInput").ap()
    wqt = nc.dram_tensor("wqt", [DIM, DIM], F32R, kind="ExternalInput").ap()
    wkt = nc.dram_tensor("wkt", [DIM, DIM], F32R, kind="ExternalInput").ap()
    wvt = nc.dram_tensor("wvt", [DIM, DIM], F32R, kind="ExternalInput").ap()
    bq_d = nc.dram_tensor("bq", [DIM], F32, kind="ExternalInput").ap()
    bk_d = nc.dram_tensor("bk", [DIM], F32, kind="ExternalInput").ap()
    cv_d = nc.dram_tensor("cross_vision", [BPC, SEQ, DIM], F32, kind="ExternalOutput").ap()
    ct_d = nc.dram_tensor("cross_text", [BPC, SEQ, DIM], F32, kind="ExternalOutput").ap()

    with tile.TileContext(nc) as tc:
        pools = []
        import contextlib
        with contextlib.ExitStack() as ctx:
            def sp(name, bufs):
                return ctx.enter_context(tc.tile_pool(name=name, bufs=bufs))

            p_act = sp("act", 1)
            p_kt = sp("kt", 1)
            p_qt = sp("qt", 1)
            p_vv = sp("vv", 1)
            p_e = sp("e", 1)
            p_etb = sp("etb", 1)
            p_wc = sp("wc", 2)
            p_wr = sp("wr", 3)
            p_in = sp("xin", 2)
            p_cvs = sp("cvs", 2)
            p_cts = sp("cts", 2)
            p_rp = sp("rp", 4)
            p_rv = sp("rv", 2)
            p_sm = sp("sm", 1)
            pp_t = ctx.enter_context(
                tc.tile_pool(name="pp_t", bufs=2, space=bass.MemorySpace.PSUM))
            pp_mm = ctx.enter_context(
                tc.tile_pool(name="pp_mm", bufs=6, space=bass.MemorySpace.PSUM))

            ident = p_sm.tile([P, P], F32, name="ident")
            make_identity(nc, ident)
            bq_sb = p_sm.tile([P, NT], F32, name="bq_sb")
            nc.sync.dma_start(out=bq_sb, in_=bq_d.rearrange("(eo ei) -> ei eo", ei=P))
            bk_sb = p_sm.tile([P, NT], F32, name="bk_sb")
            nc.sync.dma_start(out=bk_sb, in_=bk_d.rearrange("(eo ei) -> ei eo", ei=P))

            pools = (p_act, p_kt, p_qt, p_vv, p_e, p_etb, p_wc, p_wr, p_in,
                     p_cvs, p_cts, p_rp, p_rv, pp_t, pp_mm)
            for b in range(BPC):
                _emit(tc, ident, vis, txt, wqt, wkt, wvt, bq_sb, bk_sb,
                      cv_d, ct_d, pools, b)
    nc.compile()
    return nc


_NC_CACHE = None


def _get_nc():
    global _NC_CACHE
    if _NC_CACHE is None:
        _NC_CACHE = build_nc()
    return _NC_CACHE


def make_in_maps(vision_repr, text_repr, Wq, bq, Wk, bk, Wv, bv):
    s = 1.0 / np.sqrt(np.float32(DIM))
    wqt = np.ascontiguousarray(np.asarray(Wq, np.float32).T * s)
    wkt = np.ascontiguousarray(np.asarray(Wk, np.float32).T)
    wvt = np.ascontiguousarray(np.asarray(Wv, np.float32).T)
    bq_s = np.asarray(bq, np.float32) * s
    bk_ = np.asarray(bk, np.float32)
    vis = np.asarray(vision_repr, np.float32)
    txt = np.asarray(text_repr, np.float32)
    in_maps = []
    for c in range(N_CORES):
        in_maps.append({
            "vision": vis[c * BPC:(c + 1) * BPC],
            "text": txt[c * BPC:(c + 1) * BPC],
            "wqt": wqt, "wkt": wkt, "wvt": wvt,
            "bq": bq_s, "bk": bk_,
        })
    return in_maps


def kernel(vision_repr, text_repr, Wq, bq, Wk, bk, Wv, bv):
    from concourse.bass_utils import run_bass_kernel_spmd

    nc = _get_nc()
    in_maps = make_in_maps(vision_repr, text_repr, Wq, bq, Wk, bk, Wv, bv)
    res = run_bass_kernel_spmd(nc, in_maps, list(range(N_CORES))).results
    cv = np.concatenate([r_["cross_vision"] for r_ in res], axis=0)
    ct = np.concatenate([r_["cross_text"] for r_ in res], axis=0)
    cv = cv + np.asarray(bv, np.float32)[None, None, :]
    return cv, ct


# revision 6
# speedup vs baseline: 1.0683x; 1.0683x over previous
"""Cross-attention layer (vision<->text) on 8 Trainium2 NeuronCores.

Problem: B=16, Sv=St=1024, D=1024, fp32.
  q = vision @ Wq.T + bq            [B,Sv,D]
  k = text   @ Wk.T + bk            [B,St,D]
  v = text   @ Wv.T + bv            [B,St,D]
  scores = q @ k.T / sqrt(D)        [B,Sv,St]
  attn = softmax(scores, -1)
  cross_vision = attn @ v           [B,Sv,D]
  cross_text   = attn.T @ vision    [B,St,D]

Sharding: pure data-parallel over batch, 2 items per core, no collectives.

Per-core kernel design (per batch item):
  - Host pre-transposes weights: wqt = Wq.T/sqrt(D) [d,e], wkt = Wk.T, wvt = Wv.T.
    The 1/sqrt(D) is folded into wqt/bq. bv is added on the host after gather
    (attn rows sum to 1, so attn @ (v0 + bv) = attn @ v0 + bv, exact).
  - On-chip PE transposes build VT[d,s] and TT[d,t] from the natural-layout
    activations, half the seq dim at a time (SBUF economy).
  - QT[e,s] = wqt.T @ VT, KT[e,t] = wkt.T @ TT (weight tile stationary),
    Vv[t,d'] = TT.T @ wvt (TT tile stationary). All matmuls run as float32r
    (fp32 bits, PE truncates to ~fp22: full-rate at N>=512, ~2^-12 rel err).
  - S[s,t] = QT.T @ KT per 128-row s-tile; E = exp(S) straight out of PSUM on
    the ACT engine with accum_out producing row sums (scores are O(+-6), no
    max-subtraction needed for fp32 exp). rinv = 1/rowsum.
  - cross_vision s-tile: PE-transpose E row-block -> ET blocks, then
    CV = ET.T @ Vv accumulated over t-tiles, scaled by rinv at PSUM evac.
  - E is then scaled in-place by rinv (making attn rows), and
    cross_text = E.T @ V accumulated over s-tiles with raw V streamed back in.
"""

import sys

import numpy as np

if "/opt/trn_rl_repo" not in sys.path:
    sys.path.insert(0, "/opt/trn_rl_repo")

import concourse.bass as bass
import concourse.tile as tile
from concourse import bacc
from concourse import mybir

P = 128
B, SEQ, DIM = 16, 1024, 1024
N_CORES = 8
BPC = B // N_CORES  # batch items per core
NT = DIM // P  # 8 tiles of 128 along d/e
F32 = mybir.dt.float32
F32R = mybir.dt.float32r
AF = mybir.ActivationFunctionType
H = 512  # half of a seq dim / PSUM-bank-sized chunk


def _emit(tc, ident, vis, txt, wqt, wkt, wvt, bq_sb, bk_sb, cv_d, ct_d, pools, b):
    nc = tc.nc
    (p_act, p_kt, p_qt, p_vv, p_e, p_etb, p_wc, p_wr, p_in, p_cvs, p_cts, p_rp,
     p_rv, pp_t, pp_mm) = pools

    # Tiles consumed by fp32r matmuls are declared float32r: the BIR verifier
    # requires fp32r matmul inputs to be produced as fp32r (DVE/ACT round on
    # write; DMA-fed weight tiles are fp32r end-to-end from fp32r DRAM).
    kt = p_kt.tile([P, NT, SEQ], F32R, name="kt", tag="kt")
    vv = p_vv.tile([P, NT, SEQ], F32R, name="vv", tag="vv")
    qt = p_qt.tile([P, NT, SEQ], F32R, name="qt", tag="qt")

    def prep_half(src_d, h):
        """Transpose 4 seq-row-tiles of src into actT[d_in, d_out, local_seq]."""
        actT = p_act.tile([P, NT, H], F32R, name="actT", tag="actT")
        for l in range(4):
            rt = h * 4 + l
            tin = p_in.tile([P, DIM], F32R, name="tin", tag="xin")
            nc.sync.dma_start(out=tin, in_=src_d[b, rt * P:(rt + 1) * P, :].bitcast(F32R))
            for dg in range(2):  # two 4-transpose groups, each one PSUM bank
                tp4 = pp_t.tile([P, 4, P], F32R, name="tp4", tag="tp4")
                for j in range(4):
                    do = dg * 4 + j
                    nc.tensor.matmul(
                        tp4[:, j, :], tin[:, do * P:(do + 1) * P], ident,
                        is_transpose=True, start=(j == 0), stop=(j == 3),
                        skip_group_check=True,
                    )
                nc.vector.tensor_copy(actT[:, dg * 4:(dg + 1) * 4, l * P:(l + 1) * P], tp4)
        return actT

    def proj_half(w_d, bias_col, actT, out_sb, h, on_vector):
        """out_sb[:, eo, h*H:+H] = sum_do w[do,eo].T @ actT[:, do, :] (+bias)."""
        for eo in range(NT):
            wc = p_wc.tile([P, NT, P], F32R, name="wc", tag="wc")
            nc.gpsimd.dma_start(
                out=wc,
                in_=w_d[:, eo * P:(eo + 1) * P].rearrange("(do di) e -> di do e", di=P),
            )
            ps = pp_mm.tile([P, H], F32, name="ps_proj", tag="mm")
            for do in range(NT):
                nc.tensor.matmul(ps, wc[:, do, :], actT[:, do, :],
                                 start=(do == 0), stop=(do == NT - 1))
            dst = out_sb[:, eo, h * H:(h + 1) * H]
            if on_vector:
                nc.vector.tensor_scalar_add(dst, ps, scalar1=bias_col[:, eo:eo + 1])
            else:
                nc.scalar.add(dst, ps, add=bias_col[:, eo:eo + 1])

    def vv_half(actT, h):
        """vv[t-tiles of half h, :] = sum_do TT[:,do,t].T @ wvt[do-rows, :]."""
        for dc in range(2):
            pss = [pp_mm.tile([P, H], F32, name=f"ps_vv{i}", tag="mm") for i in range(4)]
            for do in range(NT):
                wr = p_wr.tile([P, H], F32R, name="wr", tag="wr")
                nc.gpsimd.dma_start(out=wr, in_=wvt[do * P:(do + 1) * P, dc * H:(dc + 1) * H])
                for l in range(4):
                    nc.tensor.matmul(pss[l], actT[:, do, l * P:(l + 1) * P], wr,
                                     start=(do == 0), stop=(do == NT - 1))
            for l in range(4):
                tt = h * 4 + l
                nc.scalar.copy(vv[:, tt, dc * H:(dc + 1) * H], pss[l])

    # ---- phases A-C: text -> TT -> KT, Vv (per t-half) ----
    for h in range(2):
        actT = prep_half(txt, h)
        proj_half(wkt, bk_sb, actT, kt, h, on_vector=False)
        vv_half(actT, h)

    # ---- phases D-E: vision -> VT -> QT (per s-half) ----
    for h in range(2):
        actV = prep_half(vis, h)
        proj_half(wqt, bq_sb, actV, qt, h, on_vector=True)

    # ---- phase F: scores, softmax, cross_vision (per s-tile) ----
    e_sb = p_e.tile([P, NT, SEQ], F32R, name="e_sb", tag="e")
    rinv = p_rv.tile([P, NT], F32, name="rinv", tag="rinv")
    for so in range(NT):
        rp = p_rp.tile([P, 2], F32, name="rp", tag="rp")
        for tc_ in range(2):
            ps = pp_mm.tile([P, H], F32, name="ps_s", tag="mm")
            for eo in range(NT):
                nc.tensor.matmul(ps, qt[:, eo, so * P:(so + 1) * P],
                                 kt[:, eo, tc_ * H:(tc_ + 1) * H],
                                 start=(eo == 0), stop=(eo == NT - 1))
            nc.scalar.activation(out=e_sb[:, so, tc_ * H:(tc_ + 1) * H], in_=ps,
                                 func=AF.Exp, accum_out=rp[:, tc_:tc_ + 1])
        rsum = p_rp.tile([P, 1], F32, name="rsum", tag="rsum")
        nc.vector.tensor_add(rsum, rp[:, 0:1], rp[:, 1:2])
        nc.vector.reciprocal(rinv[:, so:so + 1], rsum)

        # ET blocks for this s-tile (transpose the *unnormalized* E row-block)
        etb = p_etb.tile([P, NT, P], F32R, name="etb", tag="etb")
        for tg in range(2):
            tp4 = pp_t.tile([P, 4, P], F32R, name="tp4e", tag="tp4")
            for j in range(4):
                tt = tg * 4 + j
                nc.tensor.matmul(tp4[:, j, :], e_sb[:, so, tt * P:(tt + 1) * P], ident,
                                 is_transpose=True, start=(j == 0), stop=(j == 3),
                                 skip_group_check=True)
            nc.vector.tensor_copy(etb[:, tg * 4:(tg + 1) * 4, :], tp4)

        # normalize this E row-block in place (for cross_text later)
        nc.vector.tensor_scalar_mul(e_sb[:, so, :], e_sb[:, so, :],
                                    scalar1=rinv[:, so:so + 1])

        # cross_vision[s-tile] = rinv * (ET.T @ Vv)
        cvs = p_cvs.tile([P, DIM], F32, name="cvs", tag="cvs")
        for dc in range(2):
            ps = pp_mm.tile([P, H], F32, name="ps_cv", tag="mm")
            for tt in range(NT):
                nc.tensor.matmul(ps, etb[:, tt, :], vv[:, tt, dc * H:(dc + 1) * H],
                                 start=(tt == 0), stop=(tt == NT - 1))
            nc.scalar.mul(cvs[:, dc * H:(dc + 1) * H], ps, mul=rinv[:, so:so + 1])
        nc.scalar.dma_start(out=cv_d[b, so * P:(so + 1) * P, :], in_=cvs)

    # ---- phase H: cross_text = E'.T @ V (E' already rinv-scaled) ----
    for dc in range(2):
        for tg in range(2):
            pss = [pp_mm.tile([P, H], F32, name=f"ps_ct{i}", tag="mm") for i in range(4)]
            for so in range(NT):
                vt = p_in.tile([P, H], F32R, name="vt", tag="xin")
                nc.sync.dma_start(out=vt, in_=vis[b, so * P:(so + 1) * P, dc * H:(dc + 1) * H].bitcast(F32R))
                for i in range(4):
                    tt = tg * 4 + i
                    nc.tensor.matmul(pss[i], e_sb[:, so, tt * P:(tt + 1) * P], vt,
                                     start=(so == 0), stop=(so == NT - 1))
            for i in range(4):
                tt = tg * 4 + i
                cts = p_cts.tile([P, H], F32, name="cts", tag="cts")
                nc.vector.tensor_copy(cts, pss[i])
                nc.scalar.dma_start(out=ct_d[b, tt * P:(tt + 1) * P, dc * H:(dc + 1) * H],
                                     in_=cts)


def build_nc():
    nc = bacc.Bacc("TRN2", target_bir_lowering=False, debug=False, num_devices=N_CORES)
    vis = nc.dram_tensor("vision", [BPC, SEQ, DIM], F32, kind="ExternalInput").ap()
    txt = nc.dram_tensor("text", [BPC, SEQ, DIM], F32, kind="ExternalInput").ap()
    wqt = nc.dram_tensor("wqt", [DIM, DIM], F32R, kind="ExternalInput").ap()
    wkt = nc.dram_tensor("wkt", [DIM, DIM], F32R, kind="ExternalInput").ap()
    wvt = nc.dram_tensor("wvt", [DIM, DIM], F32R, kind="ExternalInput").ap()
    bq_d = nc.dram_tensor("bq", [DIM], F32, kind="ExternalInput").ap()
    id_d = nc.dram_tensor("ident128", [P, P], F32R, kind="ExternalInput").ap()
    bk_d = nc.dram_tensor("bk", [DIM], F32, kind="ExternalInput").ap()
    cv_d = nc.dram_tensor("cross_vision", [BPC, SEQ, DIM], F32, kind="ExternalOutput").ap()
    ct_d = nc.dram_tensor("cross_text", [BPC, SEQ, DIM], F32, kind="ExternalOutput").ap()

    with tile.TileContext(nc) as tc:
        pools = []
        import contextlib
        with contextlib.ExitStack() as ctx:
            def sp(name, bufs):
                return ctx.enter_context(tc.tile_pool(name=name, bufs=bufs))

            p_act = sp("act", 1)
            p_kt = sp("kt", 1)
            p_qt = sp("qt", 1)
            p_vv = sp("vv", 1)
            p_e = sp("e", 1)
            p_etb = sp("etb", 1)
            p_wc = sp("wc", 3)
            p_wr = sp("wr", 4)
            p_in = sp("xin", 2)
            p_cvs = sp("cvs", 2)
            p_cts = sp("cts", 2)
            p_rp = sp("rp", 4)
            p_rv = sp("rv", 2)
            p_sm = sp("sm", 1)
            pp_t = ctx.enter_context(
                tc.tile_pool(name="pp_t", bufs=2, space=bass.MemorySpace.PSUM))
            pp_mm = ctx.enter_context(
                tc.tile_pool(name="pp_mm", bufs=6, space=bass.MemorySpace.PSUM))

            ident = p_sm.tile([P, P], F32R, name="ident")
            nc.sync.dma_start(out=ident, in_=id_d)
            bq_sb = p_sm.tile([P, NT], F32, name="bq_sb")
            nc.sync.dma_start(out=bq_sb, in_=bq_d.rearrange("(eo ei) -> ei eo", ei=P))
            bk_sb = p_sm.tile([P, NT], F32, name="bk_sb")
            nc.sync.dma_start(out=bk_sb, in_=bk_d.rearrange("(eo ei) -> ei eo", ei=P))

            pools = (p_act, p_kt, p_qt, p_vv, p_e, p_etb, p_wc, p_wr, p_in,
                     p_cvs, p_cts, p_rp, p_rv, pp_t, pp_mm)
            for b in range(BPC):
                _emit(tc, ident, vis, txt, wqt, wkt, wvt, bq_sb, bk_sb,
                      cv_d, ct_d, pools, b)
    nc.compile()
    return nc


_NC_CACHE = None


def _get_nc():
    global _NC_CACHE
    if _NC_CACHE is None:
        _NC_CACHE = build_nc()
    return _NC_CACHE


def make_in_maps(vision_repr, text_repr, Wq, bq, Wk, bk, Wv, bv):
    s = 1.0 / np.sqrt(np.float32(DIM))
    wqt = np.ascontiguousarray(np.asarray(Wq, np.float32).T * s)
    wkt = np.ascontiguousarray(np.asarray(Wk, np.float32).T)
    wvt = np.ascontiguousarray(np.asarray(Wv, np.float32).T)
    bq_s = np.asarray(bq, np.float32) * s
    bk_ = np.asarray(bk, np.float32)
    vis = np.asarray(vision_repr, np.float32)
    txt = np.asarray(text_repr, np.float32)
    in_maps = []
    for c in range(N_CORES):
        in_maps.append({
            "vision": vis[c * BPC:(c + 1) * BPC],
            "text": txt[c * BPC:(c + 1) * BPC],
            "wqt": wqt, "wkt": wkt, "wvt": wvt,
            "bq": bq_s, "bk": bk_,
            "ident128": np.eye(P, dtype=np.float32),
        })
    return in_maps


def kernel(vision_repr, text_repr, Wq, bq, Wk, bk, Wv, bv):
    from concourse.bass_utils import run_bass_kernel_spmd

    nc = _get_nc()
    in_maps = make_in_maps(vision_repr, text_repr, Wq, bq, Wk, bk, Wv, bv)
    res = run_bass_kernel_spmd(nc, in_maps, list(range(N_CORES))).results
    cv = np.concatenate([r_["cross_vision"] for r_ in res], axis=0)
    ct = np.concatenate([r_["cross_text"] for r_ in res], axis=0)
    cv = cv + np.asarray(bv, np.float32)[None, None, :]
    return cv, ct


# revision 7
# speedup vs baseline: 1.0919x; 1.0220x over previous
"""Cross-attention layer (vision<->text) on 8 Trainium2 NeuronCores.

Problem: B=16, Sv=St=1024, D=1024, fp32.
  q = vision @ Wq.T + bq            [B,Sv,D]
  k = text   @ Wk.T + bk            [B,St,D]
  v = text   @ Wv.T + bv            [B,St,D]
  scores = q @ k.T / sqrt(D)        [B,Sv,St]
  attn = softmax(scores, -1)
  cross_vision = attn @ v           [B,Sv,D]
  cross_text   = attn.T @ vision    [B,St,D]

Sharding: pure data-parallel over batch, 2 items per core, no collectives.

Per-core kernel design (per batch item):
  - Host pre-transposes weights: wqt = Wq.T/sqrt(D) [d,e], wkt = Wk.T, wvt = Wv.T.
    The 1/sqrt(D) is folded into wqt/bq. bv is added on the host after gather
    (attn rows sum to 1, so attn @ (v0 + bv) = attn @ v0 + bv, exact).
  - On-chip PE transposes build VT[d,s] and TT[d,t] from the natural-layout
    activations, half the seq dim at a time (SBUF economy).
  - QT[e,s] = wqt.T @ VT, KT[e,t] = wkt.T @ TT (weight tile stationary),
    Vv[t,d'] = TT.T @ wvt (TT tile stationary). All matmuls run as float32r
    (fp32 bits, PE truncates to ~fp22: full-rate at N>=512, ~2^-12 rel err).
  - S[s,t] = QT.T @ KT per 128-row s-tile; E = exp(S) straight out of PSUM on
    the ACT engine with accum_out producing row sums (scores are O(+-6), no
    max-subtraction needed for fp32 exp). rinv = 1/rowsum.
  - cross_vision s-tile: PE-transpose E row-block -> ET blocks, then
    CV = ET.T @ Vv accumulated over t-tiles, scaled by rinv at PSUM evac.
  - E is then scaled in-place by rinv (making attn rows), and
    cross_text = E.T @ V accumulated over s-tiles with raw V streamed back in.
"""

import sys

import numpy as np

if "/opt/trn_rl_repo" not in sys.path:
    sys.path.insert(0, "/opt/trn_rl_repo")

import concourse.bass as bass
import concourse.tile as tile
from concourse import bacc
from concourse import mybir

P = 128
B, SEQ, DIM = 16, 1024, 1024
N_CORES = 8
BPC = B // N_CORES  # batch items per core
NT = DIM // P  # 8 tiles of 128 along d/e
F32 = mybir.dt.float32
F32R = mybir.dt.float32r
AF = mybir.ActivationFunctionType
H = 512  # half of a seq dim / PSUM-bank-sized chunk


def _emit(tc, ident, vis, txt, wqt, wkt, wvt, bq_sb, bk_sb, cv_d, ct_d, pools, b):
    nc = tc.nc
    (p_act, p_kt, p_qt, p_vv, p_e, p_etb, p_wc, p_wr, p_in, p_cvs, p_cts, p_rp,
     p_rv, pp_t, pp_mm) = pools

    # Tiles consumed by fp32r matmuls are declared float32r: the BIR verifier
    # requires fp32r matmul inputs to be produced as fp32r (DVE/ACT round on
    # write; DMA-fed weight tiles are fp32r end-to-end from fp32r DRAM).
    kt = p_kt.tile([P, NT, SEQ], F32R, name="kt", tag="kt")
    vv = p_vv.tile([P, NT, SEQ], F32R, name="vv", tag="vv")
    qt = p_qt.tile([P, NT, SEQ], F32R, name="qt", tag="qt")

    def prep_half(src_d, h):
        """Transpose 4 seq-row-tiles of src into actT[d_in, d_out, local_seq]."""
        actT = p_act.tile([P, NT, H], F32R, name="actT", tag="actT")
        for l in range(4):
            rt = h * 4 + l
            tin = p_in.tile([P, DIM], F32R, name="tin", tag="xin")
            nc.sync.dma_start(out=tin, in_=src_d[b, rt * P:(rt + 1) * P, :].bitcast(F32R))
            for dg in range(2):  # two 4-transpose groups, each one PSUM bank
                tp4 = pp_t.tile([P, 4, P], F32R, name="tp4", tag="tp4")
                for j in range(4):
                    do = dg * 4 + j
                    nc.tensor.matmul(
                        tp4[:, j, :], tin[:, do * P:(do + 1) * P], ident,
                        is_transpose=True, start=(j == 0), stop=(j == 3),
                        skip_group_check=True,
                    )
                if dg == 0:
                    nc.vector.tensor_copy(actT[:, 0:4, l * P:(l + 1) * P], tp4)
                else:
                    nc.scalar.copy(actT[:, 4:8, l * P:(l + 1) * P], tp4)
        return actT

    def proj_half(w_d, bias_col, actT, out_sb, h, on_vector):
        """out_sb[:, eo, h*H:+H] = sum_do w[do,eo].T @ actT[:, do, :] (+bias)."""
        for eo in range(NT):
            wc = p_wc.tile([P, NT, P], F32R, name="wc", tag="wc")
            nc.gpsimd.dma_start(
                out=wc,
                in_=w_d[:, eo * P:(eo + 1) * P].rearrange("(do di) e -> di do e", di=P),
            )
            ps = pp_mm.tile([P, H], F32, name="ps_proj", tag="mm")
            for do in range(NT):
                nc.tensor.matmul(ps, wc[:, do, :], actT[:, do, :],
                                 start=(do == 0), stop=(do == NT - 1))
            dst = out_sb[:, eo, h * H:(h + 1) * H]
            if on_vector:
                nc.vector.tensor_scalar_add(dst, ps, scalar1=bias_col[:, eo:eo + 1])
            else:
                nc.scalar.add(dst, ps, add=bias_col[:, eo:eo + 1])

    def vv_half(actT, h):
        """vv[t-tiles of half h, :] = sum_do TT[:,do,t].T @ wvt[do-rows, :]."""
        for dc in range(2):
            pss = [pp_mm.tile([P, H], F32, name=f"ps_vv{i}", tag="mm") for i in range(4)]
            for do in range(NT):
                wr = p_wr.tile([P, H], F32R, name="wr", tag="wr")
                nc.gpsimd.dma_start(out=wr, in_=wvt[do * P:(do + 1) * P, dc * H:(dc + 1) * H])
                for l in range(4):
                    nc.tensor.matmul(pss[l], actT[:, do, l * P:(l + 1) * P], wr,
                                     start=(do == 0), stop=(do == NT - 1))
            for l in range(4):
                tt = h * 4 + l
                nc.scalar.copy(vv[:, tt, dc * H:(dc + 1) * H], pss[l])

    # ---- phases A-C: text -> TT -> KT, Vv (per t-half) ----
    for h in range(2):
        actT = prep_half(txt, h)
        proj_half(wkt, bk_sb, actT, kt, h, on_vector=False)
        vv_half(actT, h)

    # ---- phases D-E: vision -> VT -> QT (per s-half) ----
    for h in range(2):
        actV = prep_half(vis, h)
        proj_half(wqt, bq_sb, actV, qt, h, on_vector=True)

    # ---- phase F: scores, softmax, cross_vision (per s-tile) ----
    # Software-pipelined: the PE queue is in-order, so the scores matmuls of
    # s-tile so+1 are emitted between exp(so) (ACT) and the E-transposes that
    # consume it -- the PE chews on scores while ACT produces E.
    e_sb = p_e.tile([P, NT, SEQ], F32R, name="e_sb", tag="e")
    rinv = p_rv.tile([P, NT], F32, name="rinv", tag="rinv")
    rps = {}

    def scores_stile(so):
        rp = p_rp.tile([P, 2], F32, name="rp", tag="rp")
        for tc_ in range(2):
            ps = pp_mm.tile([P, H], F32, name="ps_s", tag="mm")
            for eo in range(NT):
                nc.tensor.matmul(ps, qt[:, eo, so * P:(so + 1) * P],
                                 kt[:, eo, tc_ * H:(tc_ + 1) * H],
                                 start=(eo == 0), stop=(eo == NT - 1))
            nc.scalar.activation(out=e_sb[:, so, tc_ * H:(tc_ + 1) * H], in_=ps,
                                 func=AF.Exp, accum_out=rp[:, tc_:tc_ + 1])
        rps[so] = rp

    scores_stile(0)
    for so in range(NT):
        if so + 1 < NT:
            scores_stile(so + 1)
        rp = rps.pop(so)
        rsum = p_rp.tile([P, 1], F32, name="rsum", tag="rsum")
        nc.vector.tensor_add(rsum, rp[:, 0:1], rp[:, 1:2])
        nc.vector.reciprocal(rinv[:, so:so + 1], rsum)

        # ET blocks for this s-tile (transpose the *unnormalized* E row-block)
        etb = p_etb.tile([P, NT, P], F32R, name="etb", tag="etb")
        for tg in range(2):
            tp4 = pp_t.tile([P, 4, P], F32R, name="tp4e", tag="tp4")
            for j in range(4):
                tt = tg * 4 + j
                nc.tensor.matmul(tp4[:, j, :], e_sb[:, so, tt * P:(tt + 1) * P], ident,
                                 is_transpose=True, start=(j == 0), stop=(j == 3),
                                 skip_group_check=True)
            nc.vector.tensor_copy(etb[:, tg * 4:(tg + 1) * 4, :], tp4)

        # normalize this E row-block in place (for cross_text later)
        nc.vector.tensor_scalar_mul(e_sb[:, so, :], e_sb[:, so, :],
                                    scalar1=rinv[:, so:so + 1])

        # cross_vision[s-tile] = rinv * (ET.T @ Vv)
        cvs = p_cvs.tile([P, DIM], F32, name="cvs", tag="cvs")
        for dc in range(2):
            ps = pp_mm.tile([P, H], F32, name="ps_cv", tag="mm")
            for tt in range(NT):
                nc.tensor.matmul(ps, etb[:, tt, :], vv[:, tt, dc * H:(dc + 1) * H],
                                 start=(tt == 0), stop=(tt == NT - 1))
            nc.scalar.mul(cvs[:, dc * H:(dc + 1) * H], ps, mul=rinv[:, so:so + 1])
        nc.gpsimd.dma_start(out=cv_d[b, so * P:(so + 1) * P, :], in_=cvs)

    # ---- phase H: cross_text = E'.T @ V (E' already rinv-scaled) ----
    for dc in range(2):
        for tg in range(2):
            pss = [pp_mm.tile([P, H], F32, name=f"ps_ct{i}", tag="mm") for i in range(4)]
            for so in range(NT):
                vt = p_in.tile([P, H], F32R, name="vt", tag="xin")
                nc.sync.dma_start(out=vt, in_=vis[b, so * P:(so + 1) * P, dc * H:(dc + 1) * H].bitcast(F32R))
                for i in range(4):
                    tt = tg * 4 + i
                    nc.tensor.matmul(pss[i], e_sb[:, so, tt * P:(tt + 1) * P], vt,
                                     start=(so == 0), stop=(so == NT - 1))
            for i in range(4):
                tt = tg * 4 + i
                cts = p_cts.tile([P, H], F32, name="cts", tag="cts")
                nc.vector.tensor_copy(cts, pss[i])
                nc.gpsimd.dma_start(out=ct_d[b, tt * P:(tt + 1) * P, dc * H:(dc + 1) * H],
                                  in_=cts)


def build_nc():
    nc = bacc.Bacc("TRN2", target_bir_lowering=False, debug=False, num_devices=N_CORES)
    vis = nc.dram_tensor("vision", [BPC, SEQ, DIM], F32, kind="ExternalInput").ap()
    txt = nc.dram_tensor("text", [BPC, SEQ, DIM], F32, kind="ExternalInput").ap()
    wqt = nc.dram_tensor("wqt", [DIM, DIM], F32R, kind="ExternalInput").ap()
    wkt = nc.dram_tensor("wkt", [DIM, DIM], F32R, kind="ExternalInput").ap()
    wvt = nc.dram_tensor("wvt", [DIM, DIM], F32R, kind="ExternalInput").ap()
    bq_d = nc.dram_tensor("bq", [DIM], F32, kind="ExternalInput").ap()
    id_d = nc.dram_tensor("ident128", [P, P], F32R, kind="ExternalInput").ap()
    bk_d = nc.dram_tensor("bk", [DIM], F32, kind="ExternalInput").ap()
    cv_d = nc.dram_tensor("cross_vision", [BPC, SEQ, DIM], F32, kind="ExternalOutput").ap()
    ct_d = nc.dram_tensor("cross_text", [BPC, SEQ, DIM], F32, kind="ExternalOutput").ap()

    with tile.TileContext(nc) as tc:
        pools = []
        import contextlib
        with contextlib.ExitStack() as ctx:
            def sp(name, bufs):
                return ctx.enter_context(tc.tile_pool(name=name, bufs=bufs))

            p_act = sp("act", 1)
            p_kt = sp("kt", 1)
            p_qt = sp("qt", 1)
            p_vv = sp("vv", 1)
            p_e = sp("e", 1)
            p_etb = sp("etb", 1)
            p_wc = sp("wc", 3)
            p_wr = sp("wr", 4)
            p_in = sp("xin", 2)
            p_cvs = sp("cvs", 2)
            p_cts = sp("cts", 2)
            p_rp = sp("rp", 4)
            p_rv = sp("rv", 2)
            p_sm = sp("sm", 1)
            pp_t = ctx.enter_context(
                tc.tile_pool(name="pp_t", bufs=2, space=bass.MemorySpace.PSUM))
            pp_mm = ctx.enter_context(
                tc.tile_pool(name="pp_mm", bufs=6, space=bass.MemorySpace.PSUM))

            ident = p_sm.tile([P, P], F32R, name="ident")
            nc.sync.dma_start(out=ident, in_=id_d)
            bq_sb = p_sm.tile([P, NT], F32, name="bq_sb")
            nc.sync.dma_start(out=bq_sb, in_=bq_d.rearrange("(eo ei) -> ei eo", ei=P))
            bk_sb = p_sm.tile([P, NT], F32, name="bk_sb")
            nc.sync.dma_start(out=bk_sb, in_=bk_d.rearrange("(eo ei) -> ei eo", ei=P))

            pools = (p_act, p_kt, p_qt, p_vv, p_e, p_etb, p_wc, p_wr, p_in,
                     p_cvs, p_cts, p_rp, p_rv, pp_t, pp_mm)
            for b in range(BPC):
                _emit(tc, ident, vis, txt, wqt, wkt, wvt, bq_sb, bk_sb,
                      cv_d, ct_d, pools, b)
    nc.compile()
    return nc


_NC_CACHE = None


def _get_nc():
    global _NC_CACHE
    if _NC_CACHE is None:
        _NC_CACHE = build_nc()
    return _NC_CACHE


def make_in_maps(vision_repr, text_repr, Wq, bq, Wk, bk, Wv, bv):
    s = 1.0 / np.sqrt(np.float32(DIM))
    wqt = np.ascontiguousarray(np.asarray(Wq, np.float32).T * s)
    wkt = np.ascontiguousarray(np.asarray(Wk, np.float32).T)
    wvt = np.ascontiguousarray(np.asarray(Wv, np.float32).T)
    bq_s = np.asarray(bq, np.float32) * s
    bk_ = np.asarray(bk, np.float32)
    vis = np.asarray(vision_repr, np.float32)
    txt = np.asarray(text_repr, np.float32)
    in_maps = []
    for c in range(N_CORES):
        in_maps.append({
            "vision": vis[c * BPC:(c + 1) * BPC],
            "text": txt[c * BPC:(c + 1) * BPC],
            "wqt": wqt, "wkt": wkt, "wvt": wvt,
            "bq": bq_s, "bk": bk_,
            "ident128": np.eye(P, dtype=np.float32),
        })
    return in_maps


def kernel(vision_repr, text_repr, Wq, bq, Wk, bk, Wv, bv):
    from concourse.bass_utils import run_bass_kernel_spmd

    nc = _get_nc()
    in_maps = make_in_maps(vision_repr, text_repr, Wq, bq, Wk, bk, Wv, bv)
    res = run_bass_kernel_spmd(nc, in_maps, list(range(N_CORES))).results
    cv = np.concatenate([r_["cross_vision"] for r_ in res], axis=0)
    ct = np.concatenate([r_["cross_text"] for r_ in res], axis=0)
    cv = cv + np.asarray(bv, np.float32)[None, None, :]
    return cv, ct
